# revision 47
# baseline (speedup 1.0000x reference)
"""Trainium2 Bass kernel for nn_CP2_17669495456475 (dynamic-kernel deconv).

Math: out[b,c,y,x] = sum_l cos[b,l,i,j] * W[b,l,c,ky,kx],  y=8i+ky, x=8j+kx,
with W = unfold(pad(b)) * (1 - unfold(pad(mask))), K=16, S=8, crop 4.

Decomposition (per core): since K = 2*S, split ky = ry + 8*sy, kx = rx + 8*sx.
With u = i+sy, v = j+sx the whole op is ONE matmul with contraction over
(a,sy,sx,p) -> (l,sy,sx) of size 4096:

  outT[(c,ry,rx), (u,v)] = sum_{l,sy,sx} bm_block[(li+sy, lj+sx), (c,ry,rx)]
                                          * Xp[l, 1+u-sy, 1+v-sx]

where bm = pad(b)*(1-pad(mask)) laid out in 8x8 blocks (the unfold becomes
duplication-free shifted block views) and the deconv overlap-add is absorbed
into PSUM accumulation.  The mask multiply is fused on-device (DVE) on the
gathered W chunk tiles.

Sharding: 8 cores = 4 batches x 2 channel-halves (16 ch each). Full inputs in,
full output out; host does layout glue (replicate pad, block reshape, zero pad,
final crop/assembly) only.

Ramp/tail schedule (v2): the PE streams matmuls back-to-back at ~159ns once
started, so the only wins are the startup ramp and the drain tail.  Startup
DMAs are issued in strict priority order per queue (in-order queues are the
priority mechanism): scalar = mask[0:4] -> X slab0(a=0) -> slab0(a=1:8) ->
mask[4:32] -> deferred y-slabs; sync = W chunk0 in two half tiles (the first
matmul needs only m=0) then per-chunk W tiles, with chunks >=10 deferred
behind PE progress so W prefetch cannot starve the scalar queue.  The mask is
shipped bf16 so the (1-m) + mask-mul DVE chain in front of the first matmul
is short.  The tail ends with single-m and half-m output copies/DMAs so the
last HBM write trails the last matmul by ~1us instead of ~3us.
"""

import numpy as np

import concourse.bass as bass
import concourse.mybir as mybir
import concourse.tile as tile
from concourse.bass_utils import run_bass_kernel_spmd

PD = 4
C = 16              # channels per core
N_CORES = 8
# sy DESCENDING so the last chunk (phase-0 stop) has sy=0 (full rows in
# phase 0); chunk 0 is always emitted full-range (it carries start=True).
CHUNKS = [(a, sy, sx) for a in range(8) for sy in (1, 0) for sx in (0, 1)]

NV = 33
NT = 11 * NV          # N per matmul: 11 u-rows x NV v-cols


def _split_multi_sync(nc):
    """The walrus in this env allows only ONE sync-wait per instruction.
    Hoist extra waits onto same-engine InstNoOp carriers placed just before
    the owning instruction (sequential waits on one engine == AND)."""
    ctr = 0
    for f in nc.m.functions:
        for bb in f.blocks:
            insts = list(bb.instructions)
            out = []
            changed = False
            for inst in insts:
                si = inst.sync_info
                waits = list(si.on_wait) if si and si.on_wait else []
                if len(waits) > 1:
                    for w in waits[:-1]:
                        nop = mybir.InstNoOp(name=f"waitnop-{ctr}", ins=[], outs=[])
                        ctr += 1
                        nop.engine = inst.engine
                        nop.sync_info = mybir.SyncInfo(on_wait=[w], on_update=[])
                        out.append(nop)
                    si.on_wait = [waits[-1]]
                    changed = True
                out.append(inst)
            if changed:
                bb.instructions = out
    return ctr


def _build_nc():
    f32 = mybir.dt.float32
    bf16 = mybir.dt.bfloat16
    mmdt = bf16
    nc = bass.Bass(enable_partition_id=False)
    # W chunks pre-gathered host-side, partition-major: [p, ci, (c,ry,rx)].
    w4 = nc.declare_dram_parameter("w4", [128, 32, C * 64], mmdt, isOutput=False)
    # startup pack: per partition
    #   [mask chunks 0..11 (12*64) | s0a rows (13*36) | W chunk 0 (1024)]
    # -> ONE 4520B/partition DMA covers everything matmul (0,*) needs and
    # the masks for chunks 1-11, so the ts -> mask-mul -> first-matmul
    # chain starts as soon as the queue wakes.  (Splitting this across
    # both queues was tried and is SLOWER: the DMA engines are shared, so
    # parallel queue traffic halves the pack's packet rate, and the DVE
    # serializes the second pack's (1-m) ahead of the chunk-0 mask-mul.)
    XM_N = 12 * 64 + 13 * 36 + 1024
    xm0 = nc.declare_dram_parameter("xm0", [128, XM_N], bf16, isOutput=False)
    # mask for chunks 12..31
    mTb = nc.declare_dram_parameter("mTb", [128, 20, 64], bf16, isOutput=False)
    # X is y-major [p, yy, a, xx] and loads in 3 phase-aligned y-slabs:
    # phase n only reads rows [11n, 11n+13), so the first matmuls need just
    # slab 0 (rows 0..13) instead of the whole tensor.
    xp = nc.declare_dram_parameter("xp", [128, 34, 8, 36], mmdt, isOutput=False)
    # a-major copy of slab-0 rows for a=1..7 (a=0 lives in xm0)
    xp0 = nc.declare_dram_parameter("xp0", [128, 7, 13, 36], mmdt, isOutput=False)
    # out: phase-major [n, p, m, NT] so each phase writes large DMAs
    outT = nc.declare_dram_parameter("outT", [3, 128, 8, NT], f32, isOutput=True)

    with tile.TileContext(nc) as tc:
        with (
            tc.tile_pool(name="xpp", bufs=1) as xpp,
            tc.tile_pool(name="wp", bufs=1) as wp,
            tc.tile_pool(name="mp", bufs=1) as mp,
            tc.tile_pool(name="op", bufs=5) as op,
            tc.tile_pool(name="pp", bufs=8, space="PSUM") as pp,
        ):
            # ---- startup pack leads the SYNC queue (it wakes ~1us
            # earlier than scalar) -----------------------------------
            xm = mp.tile([128, XM_N], bf16, name="xm")
            nc.sync.dma_start(xm[:], xm0[:])
            mta0 = xm[:, 0:12 * 64].rearrange("p (c f) -> p c f", c=12)
            s0a = xm[:, 12 * 64:12 * 64 + 13 * 36].rearrange(
                "p (o y x) -> p o y x", o=1, y=13)
            W0 = 12 * 64 + 13 * 36
            # mta := 1 - m in place (bf16); chunk 0's sliver first so the
            # chunk-0 mask-mul (the first-matmul gate) starts immediately.
            for sl in (mta0[:, 0:1], mta0[:, 1:12]):
                nc.vector.tensor_scalar(
                    out=sl, in0=sl, scalar1=-1.0, scalar2=1.0,
                    op0=mybir.AluOpType.mult, op1=mybir.AluOpType.add,
                )
            # W chunks 1-3 lead the (otherwise idle-early) scalar queue so
            # they land in parallel with the startup pack on sync.
            wtile = {}
            wdmas = {}

            def wsingle(k, eng):
                wk = wp.tile([128, 1, 1024], mmdt, tag="ws", bufs=11,
                             name=f"wc{k}")
                wdmas[k] = eng.dma_start(wk[:], w4[:, k:k + 1, :])
                wtile[k] = (wk, 0)

            for k in (1, 2, 3):
                wsingle(k, nc.scalar)
            # a=1..4 rhs slab in its own tile, split from a=5..7, so chunk 4
            # (a=1) does not wait for the whole 819KB transfer (deps are
            # tile-granular).
            s0b1 = xpp.tile([128, 4, 13, 36], mmdt)
            nc.scalar.dma_start(s0b1[:], xp0[:, 0:4])
            s0b2 = xpp.tile([128, 3, 13, 36], mmdt)
            nc.scalar.dma_start(s0b2[:], xp0[:, 4:7])
            mta1 = mp.tile([128, 20, 64], bf16)
            nc.scalar.dma_start(mta1[:], mTb[:])
            nc.vector.tensor_scalar(
                out=mta1[:], in0=mta1[:], scalar1=-1.0, scalar2=1.0,
                op0=mybir.AluOpType.mult, op1=mybir.AluOpType.add,
            )

            def mask_ap(ci):
                return mta0[:, ci, :] if ci < 12 else mta1[:, ci - 12, :]
            # y-slabs for phases 1-2 (deferred behind PE progress below).
            # Slabs overlap by 2 rows so each phase reads within one slab.
            SLABS = ((0, 13), (11, 24), (22, 34))
            slab_dmas = [None]
            slabs = [None]
            for si, (y0, y1) in list(enumerate(SLABS))[1:]:
                st = xpp.tile([128, y1 - y0, 8, 36], mmdt, name=f"slab_{si}")
                slab_dmas.append(nc.scalar.dma_start(st[:], xp[:, y0:y1]))
                slabs.append(st)

            def rhs_ap(n, a, sy, sx, r0=0, r1=11):
                y0 = 11 * n + 1 - sy - SLABS[n][0]
                x0 = 1 - sx
                if n == 0:
                    if a < 1:
                        t, aa = s0a, a
                    elif a < 5:
                        t, aa = s0b1, a - 1
                    else:
                        t, aa = s0b2, a - 5
                    return t[:, aa, y0 + r0:y0 + r1, x0:x0 + NV]
                return slabs[n][:, y0 + r0:y0 + r1, a, x0:x0 + NV]

            # ---- PE pre-warm ------------------------------------------
            # The HAM clock gate runs the PE at K=4/8 (half clock) for the
            # first ~3.4us of activity.  Burn that window on dummy matmuls
            # over an uninitialized scratch tile while the startup DMAs are
            # still in flight, so the real stream starts at full clock.
            wdum = wp.tile([128, 512], mmdt, tag="wdum", bufs=1, name="wdum")
            nc.gpsimd.memset(wdum[:], 0)
            warm = pp.tile([128, 512], f32, tag="ps", name="warm")
            for _ in range(11):
                nc.tensor.matmul(warm[:], wdum[:, 0:128], wdum[:],
                                 start=True, stop=True)

            # ---- sync queue: W singles 4..11, then pairs ---------------
            NSINGLE = 12
            for k in range(4, NSINGLE):
                wsingle(k, nc.sync)
            for p in range((32 - NSINGLE) // 2):
                k = NSINGLE + 2 * p
                wq = wp.tile([128, 2, 1024], mmdt, tag="w",
                             bufs=(32 - NSINGLE) // 2, name=f"wpair_{p}")
                wdmas[k] = nc.sync.dma_start(wq[:], w4[:, k:k + 2, :])
                wtile[k] = (wq, 0)
                wtile[k + 1] = (wq, 1)

            # mask-muls in chunk order on DVE (all-bf16 ops)
            def wview(k):
                if k == 0:
                    return xm[:, W0:W0 + 1024]
                wt, j = wtile[k]
                return wt[:, j, :]

            for k in range(32):
                wv = wview(k).rearrange("p (c f) -> p c f", c=C)
                nc.vector.tensor_tensor(
                    out=wv, in0=wv,
                    in1=mask_ap(k)[:, None, :].broadcast_to([128, C, 64]),
                    op=mybir.AluOpType.mult,
                )

            def lhsT(ci, m):
                return wview(ci)[:, 128 * m:128 * (m + 1)]

            # Zero-row trim: xp row 0 (u=0, sy=1 in phase 0) and row 33
            # (u=32, sy=0 in phase 2) are pure zero padding; skip those
            # 33-col rows for every chunk except chunk 0 (which must cover
            # the full range since it carries the psum-initializing
            # start=True).  The trim keeps the psum dst contiguous (a u-row
            # is 33 consecutive cols).  Returns (row_lo, row_hi).
            def trim(n, ci, sy):
                if ci == 0:
                    return 0, 11
                if n == 0 and sy == 1:
                    return 1, 11
                if n == 2 and sy == 0:
                    return 0, 10
                return 0, 11

            # Phase 0 (n=0) is chunk-outer with 8 live psum groups so the PE
            # consumes W chunks as they stream (no all-32-chunks stall).
            # Phases 1-2 run m-outer (all data resident) so each group's
            # psum copy + output DMA overlaps the next group's matmuls.
            # Phase-2 accumulation order puts sy=1 chunks last so the
            # stop=True matmul is full-range there too.
            order2 = ([0]
                      + [ci for ci, (a, sy, sx) in enumerate(CHUNKS) if sy == 0]
                      + [ci for ci, (a, sy, sx) in enumerate(CHUNKS)
                         if sy == 1 and ci != 0])
            for n in range(3):
                # per-2m writeback tiles: dependency tracking is whole-tile,
                # so one big osb would hold every outT DMA until the LAST
                # psum copy; pair tiles let earlier halves fly mid-phase.
                def osb_pair(k):
                    return op.tile([128, 2, NT], f32, tag="o", name=f"osb_{n}_{k}")
                if n == 0:
                    pss = [pp.tile([128, NT], f32, tag="ps", name=f"ps_{n}_{i}")
                           for i in range(8)]
                    mm0 = {}
                    for ci, (a, sy, sx) in enumerate(CHUNKS):
                        r0, r1 = trim(n, ci, sy)
                        rhs = rhs_ap(n, a, sy, sx, r0, r1)
                        for m in range(8):
                            mm0[ci, m] = nc.tensor.matmul(
                                pss[m][:, 33 * r0:33 * r1], lhsT(ci, m), rhs,
                                start=(ci == 0), stop=(ci == 31),
                            )
                    # Defer the late W pairs / X slabs behind PE progress so
                    # the startup-critical DMAs get full DMA bandwidth during
                    # the ramp while keeping ~8 chunks of W prefetch in flight.
                    from concourse.tile_rust import add_dep_helper
                    for k in range(14, 32, 2):
                        add_dep_helper(wdmas[k].ins, mm0[k - 10, 0].ins,
                                       sync=True, reason="stream W behind PE")
                    add_dep_helper(slab_dmas[1].ins, mm0[11, 7].ins,
                                   sync=True, reason="slab1 after early phase0")
                    add_dep_helper(slab_dmas[2].ins, mm0[19, 7].ins,
                                   sync=True, reason="slab2 after mid phase0")
                    for k in range(4):
                        ot = osb_pair(k)
                        nc.vector.tensor_copy(ot[:, 0, :], pss[2 * k][:])
                        nc.vector.tensor_copy(ot[:, 1, :], pss[2 * k + 1][:])
                        nc.scalar.dma_start(outT[n, :, 2 * k:2 * k + 2], ot[:])
                else:
                    ot = None
                    corder = order2 if n == 2 else list(range(32))
                    for m in range(8):
                        if n == 2 and m == 7:
                            # drain tail: the last m-group accumulates in two
                            # u-part psums (u 0..5 | 6..10) run as two
                            # sequential chunk sweeps, so part A's copy+DMA
                            # hides under part B's matmuls and the final HBM
                            # write is only ~83KB.
                            UA = 6
                            ps7a = pp.tile([128, 33 * UA], f32, tag="ps",
                                           name="ps_2_7a")
                            for pos, ci in enumerate(corder):
                                a, sy, sx = CHUNKS[ci]
                                rhs = rhs_ap(n, a, sy, sx, 0, UA)
                                nc.tensor.matmul(
                                    ps7a[:], lhsT(ci, m), rhs,
                                    start=(pos == 0), stop=(pos == 31),
                                )
                            o7a = op.tile([128, 33 * UA], f32, tag="oh",
                                          bufs=1, name="osb_last7a")
                            nc.vector.tensor_copy(o7a[:], ps7a[:])
                            nc.scalar.dma_start(outT[n, :, 7, 0:33 * UA],
                                                o7a[:])
                            ps7b = pp.tile([128, NT - 33 * UA], f32, tag="ps",
                                           name="ps_2_7b")
                            for pos, ci in enumerate(corder):
                                a, sy, sx = CHUNKS[ci]
                                r0, r1 = trim(n, ci, sy)
                                rhs = rhs_ap(n, a, sy, sx, UA, r1)
                                nc.tensor.matmul(
                                    ps7b[:, 0:33 * (r1 - UA)], lhsT(ci, m),
                                    rhs,
                                    start=(pos == 0), stop=(pos == 31),
                                )
                            o7b = op.tile([128, NT - 33 * UA], f32, tag="oh2",
                                          bufs=1, name="osb_last7b")
                            nc.vector.tensor_copy(o7b[:], ps7b[:])
                            nc.scalar.dma_start(outT[n, :, 7, 33 * UA:NT],
                                                o7b[:])
                            continue
                        ps = pp.tile([128, NT], f32, tag="ps", name=f"ps_{n}_{m}")
                        for pos, ci in enumerate(corder):
                            a, sy, sx = CHUNKS[ci]
                            r0, r1 = trim(n, ci, sy)
                            rhs = rhs_ap(n, a, sy, sx, r0, r1)
                            nc.tensor.matmul(
                                ps[:, 33 * r0:33 * r1], lhsT(ci, m), rhs,
                                start=(pos == 0), stop=(pos == 31),
                            )
                        if n == 2 and m == 6:
                            o1 = op.tile([128, 1, NT], f32, tag="os",
                                         bufs=1, name="osb_last6")
                            nc.vector.tensor_copy(o1[:, 0, :], ps[:])
                            nc.scalar.dma_start(outT[n, :, 6:7], o1[:])
                            continue
                        if m % 2 == 0:
                            ot = osb_pair(m // 2)
                        nc.vector.tensor_copy(ot[:, m % 2, :], ps[:])
                        if m % 2 == 1:
                            nc.scalar.dma_start(
                                outT[n, :, m - 1:m + 1], ot[:])

    _split_multi_sync(nc)
    return nc


def _host_prep(b_ch, mask_b, cos_b):
    """b_ch (16,256,256) f32, mask_b (256,256) f32, cos_b (1024,32,32) f32
    -> dict of device inputs (layout/gather glue only)."""
    import ml_dtypes
    bpad = np.pad(b_ch, ((0, 0), (PD, PD), (PD, PD)), mode="edge")
    mpad = np.pad(mask_b, ((PD, PD), (PD, PD)), mode="edge")
    # block layout [bi*33+bj, (c,ry,rx)]
    bT = bpad.reshape(C, 33, 8, 33, 8).transpose(1, 3, 0, 2, 4).reshape(33 * 33, C * 64)
    mTb = mpad.reshape(33, 8, 33, 8).transpose(0, 2, 1, 3).reshape(33 * 33, 64)
    # unfold-as-shifted-block-views: chunk (a,sy,sx), partition p=32*pi+pj
    # reads block row (4a+pi+sy)*33 + (pj+sx).  Pre-gather partition-major.
    pi, pj = np.arange(4)[:, None], np.arange(32)[None, :]
    rows = np.stack([((4 * a + pi + sy) * 33 + (pj + sx)).reshape(128)
                     for (a, sy, sx) in CHUNKS], axis=1)        # [128, 32]
    w4 = np.ascontiguousarray(bT[rows]).astype(ml_dtypes.bfloat16)  # [128,32,1024]
    mT = mTb[rows].astype(ml_dtypes.bfloat16)                    # [128,32,64]
    xp = np.zeros((1024, 34, 36), np.float32)
    xp[:, 1:33, 1:33] = cos_b
    # [l=128a+p, yy, xx] -> [p, yy, a, xx]; plus an a-major slab-0 copy
    xpb = xp.reshape(8, 128, 34, 36).astype(ml_dtypes.bfloat16)
    xp0 = np.ascontiguousarray(xpb[1:8, :, 0:13, :].transpose(1, 0, 2, 3))
    s0a = xpb[0, :, 0:13, :].reshape(128, 13 * 36)
    xm0 = np.ascontiguousarray(
        np.concatenate([mT[:, 0:12].reshape(128, 12 * 64), s0a,
                        w4[:, 0, :]], axis=1))
    mTb_ = np.ascontiguousarray(mT[:, 12:32])
    xp = np.ascontiguousarray(xpb.transpose(1, 2, 0, 3))
    return {"w4": w4, "xm0": xm0, "mTb": mTb_, "xp": xp, "xp0": xp0}


def _unshard(outT):
    # outT [3, 128, 8, 11*NV] -> [(c,ry,rx)=128m+p, u=11n+u', v] -> (16,256,256)
    outT = np.asarray(outT, dtype=np.float32)
    t = outT.reshape(3, 128, 8, 11, NV).transpose(2, 1, 0, 3, 4).reshape(1024, 33, NV)
    t = t[:, :, :33].reshape(C, 8, 8, 33, 33).transpose(0, 3, 1, 4, 2)
    return t.reshape(C, 264, 264)[:, 4:260, 4:260]


_RUN_KW = {}   # test harness may inject e.g. trace=True
_LAST_RESULTS = [None]
_NC_CACHE = {}


def _get_nc():
    nc = _NC_CACHE.get("v3")
    if nc is None:
        nc = _NC_CACHE["v3"] = _build_nc()
    return nc


def kernel(cos_similar, b, mask):
    cos_similar = np.ascontiguousarray(np.asarray(cos_similar, dtype=np.float32))
    b = np.ascontiguousarray(np.asarray(b, dtype=np.float32))
    mask = np.ascontiguousarray(np.asarray(mask, dtype=np.float32))

    in_maps = []
    for core in range(N_CORES):
        batch, half = core // 2, core % 2
        ch0 = C * half
        in_maps.append(_host_prep(
            b[batch, ch0:ch0 + C], mask[batch, 0], cos_similar[batch]))

    nc = _get_nc()
    res = run_bass_kernel_spmd(nc, in_maps, list(range(N_CORES)), **_RUN_KW)
    _LAST_RESULTS[0] = res

    out = np.empty((4, 32, 256, 256), np.float32)
    for core in range(N_CORES):
        batch, half = core // 2, core % 2
        ch0 = C * half
        out[batch, ch0:ch0 + C] = _unshard(res.results[core]["outT"])
    return out


# revision 57
# speedup vs baseline: 1.0048x; 1.0048x over previous
"""Trainium2 Bass kernel for nn_CP2_17669495456475 (dynamic-kernel deconv).

Math: out[b,c,y,x] = sum_l cos[b,l,i,j] * W[b,l,c,ky,kx],  y=8i+ky, x=8j+kx,
with W = unfold(pad(b)) * (1 - unfold(pad(mask))), K=16, S=8, crop 4.

Decomposition (per core): since K = 2*S, split ky = ry + 8*sy, kx = rx + 8*sx.
With u = i+sy, v = j+sx the whole op is ONE matmul with contraction over
(a,sy,sx,p) -> (l,sy,sx) of size 4096:

  outT[(c,ry,rx), (u,v)] = sum_{l,sy,sx} bm_block[(li+sy, lj+sx), (c,ry,rx)]
                                          * Xp[l, 1+u-sy, 1+v-sx]

where bm = pad(b)*(1-pad(mask)) laid out in 8x8 blocks (the unfold becomes
duplication-free shifted block views) and the deconv overlap-add is absorbed
into PSUM accumulation.  The mask multiply is fused on-device (DVE) on the
gathered W chunk tiles.

Sharding: 8 cores = 4 batches x 2 channel-halves (16 ch each). Full inputs in,
full output out; host does layout glue (replicate pad, block reshape, zero pad,
final crop/assembly) only.

Ramp/tail schedule (v2): the PE streams matmuls back-to-back at ~159ns once
started, so the only wins are the startup ramp and the drain tail.  Startup
DMAs are issued in strict priority order per queue (in-order queues are the
priority mechanism): scalar = mask[0:4] -> X slab0(a=0) -> slab0(a=1:8) ->
mask[4:32] -> deferred y-slabs; sync = W chunk0 in two half tiles (the first
matmul needs only m=0) then per-chunk W tiles, with chunks >=10 deferred
behind PE progress so W prefetch cannot starve the scalar queue.  The mask is
shipped bf16 so the (1-m) + mask-mul DVE chain in front of the first matmul
is short.  The tail ends with single-m and half-m output copies/DMAs so the
last HBM write trails the last matmul by ~1us instead of ~3us.
"""

import numpy as np

import concourse.bass as bass
import concourse.mybir as mybir
import concourse.tile as tile
from concourse.bass_utils import run_bass_kernel_spmd

PD = 4
C = 16              # channels per core
N_CORES = 8
# sy DESCENDING so the last chunk (phase-0 stop) has sy=0 (full rows in
# phase 0); chunk 0 is always emitted full-range (it carries start=True).
CHUNKS = [(a, sy, sx) for a in range(8) for sy in (1, 0) for sx in (0, 1)]

NV = 33
NT = 11 * NV          # N per matmul: 11 u-rows x NV v-cols


def _split_multi_sync(nc):
    """The walrus in this env allows only ONE sync-wait per instruction.
    Hoist extra waits onto same-engine InstNoOp carriers placed just before
    the owning instruction (sequential waits on one engine == AND)."""
    ctr = 0
    for f in nc.m.functions:
        for bb in f.blocks:
            insts = list(bb.instructions)
            out = []
            changed = False
            for inst in insts:
                si = inst.sync_info
                waits = list(si.on_wait) if si and si.on_wait else []
                if len(waits) > 1:
                    for w in waits[:-1]:
                        nop = mybir.InstNoOp(name=f"waitnop-{ctr}", ins=[], outs=[])
                        ctr += 1
                        nop.engine = inst.engine
                        nop.sync_info = mybir.SyncInfo(on_wait=[w], on_update=[])
                        out.append(nop)
                    si.on_wait = [waits[-1]]
                    changed = True
                out.append(inst)
            if changed:
                bb.instructions = out
    return ctr


def _build_nc():
    f32 = mybir.dt.float32
    bf16 = mybir.dt.bfloat16
    mmdt = bf16
    nc = bass.Bass(enable_partition_id=False)
    # W chunks pre-gathered host-side, partition-major: [p, ci, (c,ry,rx)].
    w4 = nc.declare_dram_parameter("w4", [128, 32, C * 64], mmdt, isOutput=False)
    # startup pack: per partition
    #   [mask chunks 0..11 (12*64) | s0a rows (13*36) | W chunk 0 (1024)]
    # -> ONE 4520B/partition DMA covers everything matmul (0,*) needs and
    # the masks for chunks 1-11, so the ts -> mask-mul -> first-matmul
    # chain starts as soon as the queue wakes.  (Splitting this across
    # both queues was tried and is SLOWER: the DMA engines are shared, so
    # parallel queue traffic halves the pack's packet rate, and the DVE
    # serializes the second pack's (1-m) ahead of the chunk-0 mask-mul.)
    XM_N = 64 + 13 * 36 + 1024
    xm0 = nc.declare_dram_parameter("xm0", [128, XM_N], bf16, isOutput=False)
    # mask for chunks 1..11 (follows the pack on the same queue) and 12..31
    mTa = nc.declare_dram_parameter("mTa", [128, 11, 64], bf16, isOutput=False)
    mTb = nc.declare_dram_parameter("mTb", [128, 20, 64], bf16, isOutput=False)
    # X is y-major [p, yy, a, xx] and loads in 3 phase-aligned y-slabs:
    # phase n only reads rows [11n, 11n+13), so the first matmuls need just
    # slab 0 (rows 0..13) instead of the whole tensor.
    xp = nc.declare_dram_parameter("xp", [128, 34, 8, 36], mmdt, isOutput=False)
    # a-major copy of slab-0 rows for a=1..7 (a=0 lives in xm0)
    xp0 = nc.declare_dram_parameter("xp0", [128, 7, 13, 36], mmdt, isOutput=False)
    # out: phase-major [n, p, m, NT] so each phase writes large DMAs
    outT = nc.declare_dram_parameter("outT", [3, 128, 8, NT], f32, isOutput=True)

    with tile.TileContext(nc) as tc:
        with (
            tc.tile_pool(name="xpp", bufs=1) as xpp,
            tc.tile_pool(name="wp", bufs=1) as wp,
            tc.tile_pool(name="mp", bufs=1) as mp,
            tc.tile_pool(name="op", bufs=5) as op,
            tc.tile_pool(name="pp", bufs=8, space="PSUM") as pp,
        ):
            # ---- startup pack leads the SYNC queue (it wakes ~1us
            # earlier than scalar) -----------------------------------
            xm = mp.tile([128, XM_N], bf16, name="xm")
            nc.sync.dma_start(xm[:], xm0[:])
            s0a = xm[:, 64:64 + 13 * 36].rearrange(
                "p (o y x) -> p o y x", o=1, y=13)
            W0 = 64 + 13 * 36
            # masks for chunks 1-11 follow on the SAME queue (needed ~2us
            # after the pack; keeping them out of the pack shortens the
            # first-matmul chain by ~0.5us)
            mta0 = mp.tile([128, 11, 64], bf16, name="mta0")
            nc.sync.dma_start(mta0[:], mTa[:])
            # m := 1 - m in place (bf16); chunk 0's sliver first so the
            # chunk-0 mask-mul (the first-matmul gate) starts immediately.
            nc.vector.tensor_scalar(
                out=xm[:, 0:64], in0=xm[:, 0:64], scalar1=-1.0, scalar2=1.0,
                op0=mybir.AluOpType.mult, op1=mybir.AluOpType.add,
            )
            # chunk-0 mask-mul emitted BEFORE the other (1-m) ops so the DVE
            # never serializes the first-matmul gate behind later transfers
            wv0 = xm[:, W0:W0 + 1024].rearrange("p (c f) -> p c f", c=C)
            nc.vector.tensor_tensor(
                out=wv0, in0=wv0,
                in1=xm[:, 0:64][:, None, :].broadcast_to([128, C, 64]),
                op=mybir.AluOpType.mult,
            )
            nc.vector.tensor_scalar(
                out=mta0[:], in0=mta0[:], scalar1=-1.0, scalar2=1.0,
                op0=mybir.AluOpType.mult, op1=mybir.AluOpType.add,
            )
            # W chunks 1-3 lead the (otherwise idle-early) scalar queue so
            # they land in parallel with the startup pack on sync.
            wtile = {}
            wdmas = {}

            def wsingle(k, eng):
                wk = wp.tile([128, 1, 1024], mmdt, tag="ws", bufs=11,
                             name=f"wc{k}")
                wdmas[k] = eng.dma_start(wk[:], w4[:, k:k + 1, :])
                wtile[k] = (wk, 0)

            for k in (1, 2, 3):
                wsingle(k, nc.scalar)
            # a=1..4 rhs slab in its own tile, split from a=5..7, so chunk 4
            # (a=1) does not wait for the whole 819KB transfer (deps are
            # tile-granular).
            s0b1 = xpp.tile([128, 4, 13, 36], mmdt)
            nc.scalar.dma_start(s0b1[:], xp0[:, 0:4])
            s0b2 = xpp.tile([128, 3, 13, 36], mmdt)
            nc.scalar.dma_start(s0b2[:], xp0[:, 4:7])
            mta1 = mp.tile([128, 20, 64], bf16)
            nc.scalar.dma_start(mta1[:], mTb[:])
            nc.vector.tensor_scalar(
                out=mta1[:], in0=mta1[:], scalar1=-1.0, scalar2=1.0,
                op0=mybir.AluOpType.mult, op1=mybir.AluOpType.add,
            )

            def mask_ap(ci):
                if ci == 0:
                    return xm[:, 0:64]
                if ci < 12:
                    return mta0[:, ci - 1, :]
                return mta1[:, ci - 12, :]
            # y-slabs for phases 1-2 (deferred behind PE progress below).
            # Slabs overlap by 2 rows so each phase reads within one slab.
            SLABS = ((0, 13), (11, 24), (22, 34))
            slab_dmas = [None]
            slabs = [None]
            for si, (y0, y1) in list(enumerate(SLABS))[1:]:
                st = xpp.tile([128, y1 - y0, 8, 36], mmdt, name=f"slab_{si}")
                slab_dmas.append(nc.scalar.dma_start(st[:], xp[:, y0:y1]))
                slabs.append(st)

            def rhs_ap(n, a, sy, sx, r0=0, r1=11):
                y0 = 11 * n + 1 - sy - SLABS[n][0]
                x0 = 1 - sx
                if n == 0:
                    if a < 1:
                        t, aa = s0a, a
                    elif a < 5:
                        t, aa = s0b1, a - 1
                    else:
                        t, aa = s0b2, a - 5
                    return t[:, aa, y0 + r0:y0 + r1, x0:x0 + NV]
                return slabs[n][:, y0 + r0:y0 + r1, a, x0:x0 + NV]

            # ---- PE pre-warm ------------------------------------------
            # The HAM clock gate runs the PE at K=4/8 (half clock) for the
            # first ~3.4us of activity.  Burn that window on dummy matmuls
            # over an uninitialized scratch tile while the startup DMAs are
            # still in flight, so the real stream starts at full clock.
            wdum = wp.tile([128, 512], mmdt, tag="wdum", bufs=1, name="wdum")
            nc.gpsimd.memset(wdum[:], 0)
            warm = pp.tile([128, 512], f32, tag="ps", name="warm")
            for _ in range(10):
                nc.tensor.matmul(warm[:], wdum[:, 0:128], wdum[:],
                                 start=True, stop=True)

            # ---- sync queue: W singles 4..11, then pairs ---------------
            NSINGLE = 12
            for k in range(4, NSINGLE):
                wsingle(k, nc.sync)
            for p in range((32 - NSINGLE) // 2):
                k = NSINGLE + 2 * p
                wq = wp.tile([128, 2, 1024], mmdt, tag="w",
                             bufs=(32 - NSINGLE) // 2, name=f"wpair_{p}")
                wdmas[k] = nc.sync.dma_start(wq[:], w4[:, k:k + 2, :])
                wtile[k] = (wq, 0)
                wtile[k + 1] = (wq, 1)

            # mask-muls in chunk order on DVE (all-bf16 ops)
            def wview(k):
                if k == 0:
                    return xm[:, W0:W0 + 1024]
                wt, j = wtile[k]
                return wt[:, j, :]

            for k in range(1, 32):
                wv = wview(k).rearrange("p (c f) -> p c f", c=C)
                nc.vector.tensor_tensor(
                    out=wv, in0=wv,
                    in1=mask_ap(k)[:, None, :].broadcast_to([128, C, 64]),
                    op=mybir.AluOpType.mult,
                )

            def lhsT(ci, m):
                return wview(ci)[:, 128 * m:128 * (m + 1)]

            # Zero-row trim: xp row 0 (u=0, sy=1 in phase 0) and row 33
            # (u=32, sy=0 in phase 2) are pure zero padding; skip those
            # 33-col rows for every chunk except chunk 0 (which must cover
            # the full range since it carries the psum-initializing
            # start=True).  The trim keeps the psum dst contiguous (a u-row
            # is 33 consecutive cols).  Returns (row_lo, row_hi).
            def trim(n, ci, sy):
                if ci == 0:
                    return 0, 11
                if n == 0 and sy == 1:
                    return 1, 11
                if n == 2 and sy == 0:
                    return 0, 10
                return 0, 11

            # Phase 0 (n=0) is chunk-outer with 8 live psum groups so the PE
            # consumes W chunks as they stream (no all-32-chunks stall).
            # Phases 1-2 run m-outer (all data resident) so each group's
            # psum copy + output DMA overlaps the next group's matmuls.
            # Phase-2 accumulation order puts sy=1 chunks last so the
            # stop=True matmul is full-range there too.
            order2 = ([0]
                      + [ci for ci, (a, sy, sx) in enumerate(CHUNKS) if sy == 0]
                      + [ci for ci, (a, sy, sx) in enumerate(CHUNKS)
                         if sy == 1 and ci != 0])
            for n in range(3):
                # per-2m writeback tiles: dependency tracking is whole-tile,
                # so one big osb would hold every outT DMA until the LAST
                # psum copy; pair tiles let earlier halves fly mid-phase.
                def osb_pair(k):
                    return op.tile([128, 2, NT], f32, tag="o", name=f"osb_{n}_{k}")
                if n == 0:
                    pss = [pp.tile([128, NT], f32, tag="ps", name=f"ps_{n}_{i}")
                           for i in range(8)]
                    mm0 = {}
                    for ci, (a, sy, sx) in enumerate(CHUNKS):
                        r0, r1 = trim(n, ci, sy)
                        rhs = rhs_ap(n, a, sy, sx, r0, r1)
                        for m in range(8):
                            mm0[ci, m] = nc.tensor.matmul(
                                pss[m][:, 33 * r0:33 * r1], lhsT(ci, m), rhs,
                                start=(ci == 0), stop=(ci == 31),
                            )
                    # Defer the late W pairs / X slabs behind PE progress so
                    # the startup-critical DMAs get full DMA bandwidth during
                    # the ramp while keeping ~8 chunks of W prefetch in flight.
                    from concourse.tile_rust import add_dep_helper
                    for k in range(14, 32, 2):
                        add_dep_helper(wdmas[k].ins, mm0[k - 10, 0].ins,
                                       sync=True, reason="stream W behind PE")
                    add_dep_helper(slab_dmas[1].ins, mm0[11, 7].ins,
                                   sync=True, reason="slab1 after early phase0")
                    add_dep_helper(slab_dmas[2].ins, mm0[19, 7].ins,
                                   sync=True, reason="slab2 after mid phase0")
                    for k in range(4):
                        ot = osb_pair(k)
                        nc.vector.tensor_copy(ot[:, 0, :], pss[2 * k][:])
                        nc.vector.tensor_copy(ot[:, 1, :], pss[2 * k + 1][:])
                        nc.scalar.dma_start(outT[n, :, 2 * k:2 * k + 2], ot[:])
                else:
                    ot = None
                    corder = order2 if n == 2 else list(range(32))
                    for m in range(8):
                        if n == 2 and m == 7:
                            # drain tail: the last m-group accumulates in two
                            # u-part psums (u 0..5 | 6..10) run as two
                            # sequential chunk sweeps, so part A's copy+DMA
                            # hides under part B's matmuls and the final HBM
                            # write is only ~83KB.
                            UA = 6
                            ps7a = pp.tile([128, 33 * UA], f32, tag="ps",
                                           name="ps_2_7a")
                            for pos, ci in enumerate(corder):
                                a, sy, sx = CHUNKS[ci]
                                rhs = rhs_ap(n, a, sy, sx, 0, UA)
                                nc.tensor.matmul(
                                    ps7a[:], lhsT(ci, m), rhs,
                                    start=(pos == 0), stop=(pos == 31),
                                )
                            o7a = op.tile([128, 33 * UA], f32, tag="oh",
                                          bufs=1, name="osb_last7a")
                            nc.vector.tensor_copy(o7a[:], ps7a[:])
                            nc.scalar.dma_start(outT[n, :, 7, 0:33 * UA],
                                                o7a[:])
                            ps7b = pp.tile([128, NT - 33 * UA], f32, tag="ps",
                                           name="ps_2_7b")
                            for pos, ci in enumerate(corder):
                                a, sy, sx = CHUNKS[ci]
                                r0, r1 = trim(n, ci, sy)
                                rhs = rhs_ap(n, a, sy, sx, UA, r1)
                                nc.tensor.matmul(
                                    ps7b[:, 0:33 * (r1 - UA)], lhsT(ci, m),
                                    rhs,
                                    start=(pos == 0), stop=(pos == 31),
                                )
                            # final write as two column-halves on BOTH
                            # queues: the two 0.6us DMA-issue costs and the
                            # two ~42KB transfers overlap.
                            NB = NT - 33 * UA
                            HB = NB // 2
                            o7b = op.tile([128, HB], f32, tag="oh2",
                                          bufs=1, name="osb_last7b")
                            nc.vector.tensor_copy(o7b[:], ps7b[:, 0:HB])
                            nc.scalar.dma_start(
                                outT[n, :, 7, 33 * UA:33 * UA + HB], o7b[:])
                            o7c = op.tile([128, NB - HB], f32, tag="oh3",
                                          bufs=1, name="osb_last7c")
                            nc.vector.tensor_copy(o7c[:], ps7b[:, HB:NB])
                            nc.sync.dma_start(
                                outT[n, :, 7, 33 * UA + HB:NT], o7c[:])
                            continue
                        ps = pp.tile([128, NT], f32, tag="ps", name=f"ps_{n}_{m}")
                        for pos, ci in enumerate(corder):
                            a, sy, sx = CHUNKS[ci]
                            r0, r1 = trim(n, ci, sy)
                            rhs = rhs_ap(n, a, sy, sx, r0, r1)
                            nc.tensor.matmul(
                                ps[:, 33 * r0:33 * r1], lhsT(ci, m), rhs,
                                start=(pos == 0), stop=(pos == 31),
                            )
                        if n == 2 and m == 6:
                            o1 = op.tile([128, 1, NT], f32, tag="os",
                                         bufs=1, name="osb_last6")
                            nc.vector.tensor_copy(o1[:, 0, :], ps[:])
                            nc.scalar.dma_start(outT[n, :, 6:7], o1[:])
                            continue
                        if m % 2 == 0:
                            ot = osb_pair(m // 2)
                        nc.vector.tensor_copy(ot[:, m % 2, :], ps[:])
                        if m % 2 == 1:
                            nc.scalar.dma_start(
                                outT[n, :, m - 1:m + 1], ot[:])

    _split_multi_sync(nc)
    return nc


def _host_prep(b_ch, mask_b, cos_b):
    """b_ch (16,256,256) f32, mask_b (256,256) f32, cos_b (1024,32,32) f32
    -> dict of device inputs (layout/gather glue only)."""
    import ml_dtypes
    bpad = np.pad(b_ch, ((0, 0), (PD, PD), (PD, PD)), mode="edge")
    mpad = np.pad(mask_b, ((PD, PD), (PD, PD)), mode="edge")
    # block layout [bi*33+bj, (c,ry,rx)]
    bT = bpad.reshape(C, 33, 8, 33, 8).transpose(1, 3, 0, 2, 4).reshape(33 * 33, C * 64)
    mTb = mpad.reshape(33, 8, 33, 8).transpose(0, 2, 1, 3).reshape(33 * 33, 64)
    # unfold-as-shifted-block-views: chunk (a,sy,sx), partition p=32*pi+pj
    # reads block row (4a+pi+sy)*33 + (pj+sx).  Pre-gather partition-major.
    pi, pj = np.arange(4)[:, None], np.arange(32)[None, :]
    rows = np.stack([((4 * a + pi + sy) * 33 + (pj + sx)).reshape(128)
                     for (a, sy, sx) in CHUNKS], axis=1)        # [128, 32]
    w4 = np.ascontiguousarray(bT[rows]).astype(ml_dtypes.bfloat16)  # [128,32,1024]
    mT = mTb[rows].astype(ml_dtypes.bfloat16)                    # [128,32,64]
    xp = np.zeros((1024, 34, 36), np.float32)
    xp[:, 1:33, 1:33] = cos_b
    # [l=128a+p, yy, xx] -> [p, yy, a, xx]; plus an a-major slab-0 copy
    xpb = xp.reshape(8, 128, 34, 36).astype(ml_dtypes.bfloat16)
    xp0 = np.ascontiguousarray(xpb[1:8, :, 0:13, :].transpose(1, 0, 2, 3))
    s0a = xpb[0, :, 0:13, :].reshape(128, 13 * 36)
    xm0 = np.ascontiguousarray(
        np.concatenate([mT[:, 0], s0a, w4[:, 0, :]], axis=1))
    mTa_ = np.ascontiguousarray(mT[:, 1:12])
    mTb_ = np.ascontiguousarray(mT[:, 12:32])
    xp = np.ascontiguousarray(xpb.transpose(1, 2, 0, 3))
    return {"w4": w4, "xm0": xm0, "mTa": mTa_, "mTb": mTb_, "xp": xp,
            "xp0": xp0}


def _unshard(outT):
    # outT [3, 128, 8, 11*NV] -> [(c,ry,rx)=128m+p, u=11n+u', v] -> (16,256,256)
    outT = np.asarray(outT, dtype=np.float32)
    t = outT.reshape(3, 128, 8, 11, NV).transpose(2, 1, 0, 3, 4).reshape(1024, 33, NV)
    t = t[:, :, :33].reshape(C, 8, 8, 33, 33).transpose(0, 3, 1, 4, 2)
    return t.reshape(C, 264, 264)[:, 4:260, 4:260]


_RUN_KW = {}   # test harness may inject e.g. trace=True
_LAST_RESULTS = [None]
_NC_CACHE = {}


def _get_nc():
    nc = _NC_CACHE.get("v3")
    if nc is None:
        nc = _NC_CACHE["v3"] = _build_nc()
    return nc


def kernel(cos_similar, b, mask):
    cos_similar = np.ascontiguousarray(np.asarray(cos_similar, dtype=np.float32))
    b = np.ascontiguousarray(np.asarray(b, dtype=np.float32))
    mask = np.ascontiguousarray(np.asarray(mask, dtype=np.float32))

    in_maps = []
    for core in range(N_CORES):
        batch, half = core // 2, core % 2
        ch0 = C * half
        in_maps.append(_host_prep(
            b[batch, ch0:ch0 + C], mask[batch, 0], cos_similar[batch]))

    nc = _get_nc()
    res = run_bass_kernel_spmd(nc, in_maps, list(range(N_CORES)), **_RUN_KW)
    _LAST_RESULTS[0] = res

    out = np.empty((4, 32, 256, 256), np.float32)
    for core in range(N_CORES):
        batch, half = core // 2, core % 2
        ch0 = C * half
        out[batch, ch0:ch0 + C] = _unshard(res.results[core]["outT"])
    return out
